# revision 1
# baseline (speedup 1.0000x reference)
"""Trainium2 Bass kernel for hierarchical softmax tree posterior (HNet.predict).

Math: per internal node i (level-order, children 2i+1/2i+2), softmax over 2
children of Linear(x). Path probabilities multiply down a depth-12 complete
binary tree; output p [B, 4096] leaf posteriors.

Key identities used:
  softmax([l0, l1])[0] = sigmoid(l0 - l1), [1] = 1 - sigmoid(l0 - l1)
  => only the logit DIFFERENCE matters: d_j = x . (W_j0 - W_j1) + (b_j0 - b_j1)
  => one [B,64] @ [64,4095] matmul (bias folded in as a 65th contraction row),
     sigmoid on ScalarE, then multiply-down-the-tree on VectorE:
     child0 = p * s, child1 = p - child0.
     (GPSIMD offload of subtractions was measured NET-NEGATIVE: it shares an
     SBUF port with VectorE and serializes; TensorTensor cannot run on
     ScalarE on TRN2 — so the whole tree stays on the DVE.)

Sharding: batch B=8192 split across 8 cores (1024 rows each); tree params
replicated. Output [B, 4096] f32 = 128MB dominates traffic (memory-bound).
"""

import contextlib

import numpy as np

import concourse.bacc as bacc
import concourse.mybir as mybir
import concourse.tile as tile
from concourse.bass_utils import run_bass_kernel_spmd

B, D = 8192, 64
NODES = 4095          # internal nodes, level-order
LEAVES = 4096
NCORES = 8
BLOC = B // NCORES    # 1024 rows per core
KA = D + 1            # contraction dim incl. bias row
NBT = BLOC // 128     # 8 batch tiles of 128 rows

F32 = mybir.dt.float32
# float32r runs the PE at 1 cyc/row (vs 4 for exact fp32); measured end-to-end
# output error 2.4e-4 rel-to-scale. DRAM inputs are declared float32r directly
# (same bytes as f32) so no on-device cast is needed.
MM_DT = mybir.dt.float32r

# Pair-columns of the level-10/11 odd-child subtractions on GPSIMD instead of
# VectorE. Measured on HW: any GPSIMD share is slower (shared SBUF port with
# DVE serializes the engines), so these stay 0.
GP_SUB10 = 0      # of 1024
GP_SUB11 = 0      # of 2048


def _build(reps=1):
    nc = bacc.Bacc("TRN2", target_bir_lowering=False, debug=False, num_devices=NCORES)
    wdt = nc.dram_tensor("wdt", [KA, LEAVES], MM_DT, kind="ExternalInput")
    xt = nc.dram_tensor("xt", [KA, BLOC], MM_DT, kind="ExternalInput")
    out = nc.dram_tensor("out", [BLOC, LEAVES], F32, kind="ExternalOutput")

    SIG = mybir.ActivationFunctionType.Sigmoid
    IDN = mybir.ActivationFunctionType.Identity

    with tile.TileContext(nc) as tc:
        with (
            tc.tile_pool(name="const", bufs=1) as const,
            tc.tile_pool(name="pa", bufs=1) as pa,
            tc.tile_pool(name="pb", bufs=2) as pb,
            tc.tile_pool(name="ps", bufs=2, space="PSUM") as psp,
        ):
            wdt_r = const.tile([KA, LEAVES], MM_DT)
            xt_r = const.tile([KA, BLOC], MM_DT)
            nc.sync.dma_start(out=wdt_r[:], in_=wdt[:])
            nc.sync.dma_start(out=xt_r[:], in_=xt[:])

            loop = tc.For_i(0, reps, 1) if reps > 1 else contextlib.nullcontext()
            with loop:
                _emit_body(nc, tc, pa, pb, psp, wdt_r, xt_r, out, SIG, IDN)

    nc.compile()
    return nc


def _emit_body(nc, tc, pa, pb, psp, wdt_r, xt_r, out, SIG, IDN):
    # ---- phase A: nodes 0..1022 (levels 0..9) fused across all 8 batch tiles
    s_small = pa.tile([128, NBT, 1024], F32, tag="s_small")
    for bt in range(NBT):
        ps = psp.tile([128, 1024], F32, tag="ps")
        for c in range(2):
            nc.tensor.matmul(
                ps[:, c * 512:(c + 1) * 512],
                xt_r[:, bt * 128:(bt + 1) * 128],
                wdt_r[:, c * 512:(c + 1) * 512],
                start=True, stop=True,
            )
        nc.scalar.activation(out=s_small[:, bt, :], in_=ps[:], func=SIG)

    pA = pa.tile([128, NBT, 512], F32, tag="pA")
    pB = pa.tile([128, NBT, 512], F32, tag="pB")
    p10 = pa.tile([128, NBT, 1024], F32, tag="p10")
    # level 0: p1 = [s0, 1-s0]
    nc.vector.tensor_copy(pA[:, :, 0:1], s_small[:, :, 0:1])
    nc.scalar.activation(out=pA[:, :, 1:2], in_=s_small[:, :, 0:1],
                         func=IDN, bias=1.0, scale=-1.0)
    cur, other = pA, pB
    for lvl in range(1, 10):
        n = 1 << lvl
        off = n - 1
        nxt = p10 if lvl == 9 else other
        nxt4 = nxt[:, :, 0:2 * n].rearrange("p g (n two) -> p g n two", two=2)
        nc.vector.tensor_mul(nxt4[:, :, :, 0], cur[:, :, 0:n],
                             s_small[:, :, off:off + n])
        nc.vector.tensor_sub(nxt4[:, :, :, 1], cur[:, :, 0:n],
                             nxt4[:, :, :, 0])
        other, cur = cur, nxt

    # ---- phase B: nodes 1023..4094 (levels 10..11), per batch tile
    for bt in range(NBT):
        ps1 = psp.tile([128, 2048], F32, tag="ps")   # nodes 1023..3070
        for c in range(4):
            nc.tensor.matmul(
                ps1[:, c * 512:(c + 1) * 512],
                xt_r[:, bt * 128:(bt + 1) * 128],
                wdt_r[:, 1023 + c * 512:1023 + (c + 1) * 512],
                start=True, stop=True,
            )
        ps2 = psp.tile([128, 1024], F32, tag="ps")   # nodes 3071..4094
        for c in range(2):
            nc.tensor.matmul(
                ps2[:, c * 512:(c + 1) * 512],
                xt_r[:, bt * 128:(bt + 1) * 128],
                wdt_r[:, 3071 + c * 512:3071 + (c + 1) * 512],
                start=True, stop=True,
            )
        sb = pb.tile([128, 1024], F32, tag="sbig")
        nc.scalar.activation(out=sb[:], in_=ps1[:, 0:1024], func=SIG)
        # level-11 sigmoids written interleaved (sigma(+d), sigma(-d)) so the
        # last level needs only ONE DVE multiply with a step-0 broadcast of
        # p11 and a unit-stride output (replaces strided mul+sub pair).
        s11 = pb.tile([128, 4096], F32, tag="s11")
        s11v = s11.rearrange("p (n two) -> p n two", two=2)
        nc.scalar.activation(out=s11v[:, 0:1024, 0], in_=ps1[:, 1024:2048], func=SIG)
        nc.scalar.activation(out=s11v[:, 0:1024, 1], in_=ps1[:, 1024:2048], func=SIG, scale=-1.0)
        nc.scalar.activation(out=s11v[:, 1024:2048, 0], in_=ps2[:], func=SIG)
        nc.scalar.activation(out=s11v[:, 1024:2048, 1], in_=ps2[:], func=SIG, scale=-1.0)

        # level 10: p10 [*,1024] -> p11 [*,2048]; s nodes 1023..2046
        p11 = pb.tile([128, 2048], F32, tag="p11")
        p11v = p11.rearrange("p (n two) -> p n two", two=2)
        nc.vector.tensor_mul(p11v[:, :, 0], p10[:, bt, :], sb[:])
        nc.vector.tensor_sub(p11v[:, :, 1], p10[:, bt, :], p11v[:, :, 0])

        # level 11: one broadcast multiply into the output tile
        ot = pb.tile([128, 4096], F32, tag="out")
        otv = ot.rearrange("p (n two) -> p n two", two=2)
        nc.vector.tensor_mul(otv[:], p11[:].broadcast_to([128, 2048, 2]), s11v[:])

        nc.sync.dma_start(out=out[bt * 128:(bt + 1) * 128, :], in_=ot[:])


_NC_CACHE = {}


def _get_nc(reps=1):
    if reps not in _NC_CACHE:
        _NC_CACHE[reps] = _build(reps)
    return _NC_CACHE[reps]


def _prep_inputs(x, W, b):
    x = np.asarray(x, dtype=np.float32)
    W = np.asarray(W, dtype=np.float32)
    b = np.asarray(b, dtype=np.float32)
    Wd = W[:, 0, :] - W[:, 1, :]          # [4095, 64]
    bd = b[:, 0] - b[:, 1]                # [4095]
    wdt = np.zeros((KA, LEAVES), dtype=np.float32)
    wdt[:D, :NODES] = Wd.T
    wdt[D, :NODES] = bd
    xt = np.empty((KA, B), dtype=np.float32)
    xt[:D] = x.T
    xt[D] = 1.0
    in_maps = [
        {"wdt": wdt, "xt": np.ascontiguousarray(xt[:, c * BLOC:(c + 1) * BLOC])}
        for c in range(NCORES)
    ]
    return in_maps


def kernel(x, W, b):
    in_maps = _prep_inputs(x, W, b)
    nc = _get_nc()
    res = run_bass_kernel_spmd(nc, in_maps, core_ids=list(range(NCORES)))
    return np.concatenate([res.results[c]["out"] for c in range(NCORES)], axis=0)


if __name__ == "__main__":
    rng = np.random.default_rng(0)
    x = rng.standard_normal((B, D)).astype(np.float32)
    W = (rng.standard_normal((NODES, 2, D)) * 0.1).astype(np.float32)
    b = (rng.standard_normal((NODES, 2)) * 0.1).astype(np.float32)
    p = kernel(x, W, b)
    print("out", p.shape, p.dtype, "rowsum", p.sum(axis=1)[:4])



# revision 17
# speedup vs baseline: 18.6646x; 18.6646x over previous
"""Trainium2 Bass kernel for hierarchical softmax tree posterior (HNet.predict).

Math: per internal node i (level-order, children 2i+1/2i+2), softmax over 2
children of Linear(x). Path probabilities multiply down a depth-12 complete
binary tree; output p [B, 4096] leaf posteriors.

Key identities / layout tricks:
  softmax([l0,l1])[0] = sigmoid(l0-l1): only the logit DIFFERENCE matters,
  so one [B,65] @ [65,4095] matmul (bias folded as 65th row) gives all d.
  sigma(-d) = 1 - sigma(d): each tree level is mul + sub on VectorE.

  CONCAT layout: v_{l+1} = [v_l * s_l || v_l - v_l * s_l] keeps every DVE
  operand dense step-1, which (with fp16) enables the 2x_1p DVE perf mode:
  2 elem/cycle/lane vs 1 for fp32 or strided/interleaved child pairs.
  The price: the leaf axis comes out BIT-REVERSED (each level's branch bit
  becomes the MSB of the position index). The host un-permutes columns at
  gather time (and the tree-node columns of W are pre-permuted to match).

  fp16 everywhere after PSUM: halves DVE time (2x mode) AND halves the
  dominant output DMA traffic (16.8MB -> 8.4MB per core). A 2^14 scale is
  injected at the root so path products stay in fp16 normal range; host
  multiplies by 2^-14 (exact) after converting to f32.

Schedule: per-tile (128 rows) pipeline matmul -> sigmoid -> tree -> DMA so
output DMA overlaps compute from the first tile on. Levels 0..7 are batched
across tiles in two halves of 4 (amortizes DVE op overhead without gating
tile 0 on all 8 phase-A matmuls); the half-2 shallow tree is emitted after
tile 1 so it stays off the DVE critical path during ramp-up.

Sharding: batch B=8192 split across 8 cores (1024 rows each); tree params
replicated.
"""

import contextlib

import numpy as np

import concourse.bacc as bacc
import concourse.mybir as mybir
import concourse.tile as tile
from concourse.bass_utils import run_bass_kernel_spmd

B, D = 8192, 64
NODES = 4095          # internal nodes, level-order
LEAVES = 4096
DEPTH = 12
NCORES = 8
BLOC = B // NCORES    # 1024 rows per core
KA = D + 1            # contraction dim incl. bias row
NBT = BLOC // 128     # 8 batch tiles of 128 rows

C_SCALE = 2.0 ** 14   # root scale keeping fp16 path products normal

F32 = mybir.dt.float32
F16 = mybir.dt.float16
# float32r runs the PE at 1 cyc/row (vs 4 for exact fp32); DRAM inputs are
# declared float32r directly (same bytes as f32) so no on-device cast.
MM_DT = mybir.dt.float32r

# Column split of the per-tile matmul across two PSUM tiles (levels 8..11;
# levels 0..7 = cols 0..254 are matmul'd in phase A).
PS1_LO, PS1_HI = 255, 2303    # 2048 cols: 4x512 chunks
PS2_LO, PS2_HI = 2303, 4095   # 1792 cols: 512,512,512,256 chunks


def _build(reps=1):
    nc = bacc.Bacc("TRN2", target_bir_lowering=False, debug=False, num_devices=NCORES)
    wdt = nc.dram_tensor("wdt", [KA, LEAVES], MM_DT, kind="ExternalInput")
    xt = nc.dram_tensor("xt", [KA, BLOC], MM_DT, kind="ExternalInput")
    out = nc.dram_tensor("out", [BLOC, LEAVES], F16, kind="ExternalOutput")

    with tile.TileContext(nc) as tc:
        with (
            tc.tile_pool(name="const", bufs=1) as const,
            tc.tile_pool(name="pa", bufs=1) as pa,
            tc.tile_pool(name="pb", bufs=2) as pb,
            tc.tile_pool(name="ps", bufs=2, space="PSUM") as psp,
        ):
            wdt_r = const.tile([KA, LEAVES], MM_DT, tag="wdt_r")
            xt_r = const.tile([KA, BLOC], MM_DT, tag="xt_r")
            # phase-A weight cols + tile 0-3 xt first: they gate the pipeline
            nc.sync.dma_start(out=wdt_r[:, 0:256], in_=wdt[:, 0:256])
            nc.sync.dma_start(out=xt_r[:, 0:512], in_=xt[:, 0:512])
            nc.sync.dma_start(out=xt_r[:, 512:BLOC], in_=xt[:, 512:BLOC])
            nc.sync.dma_start(out=wdt_r[:, 256:LEAVES], in_=wdt[:, 256:LEAVES])

            loop = tc.For_i(0, reps, 1) if reps > 1 else contextlib.nullcontext()
            with loop:
                _emit_body(nc, tc, pa, pb, psp, wdt_r, xt_r, out)

    nc.compile()
    return nc


def _phase_a_mm(nc, psp, wdt_r, xt_r, half):
    """Matmul of tree cols 0..255 for tiles half*4..half*4+3, packed at 256
    spacing in one 4-bank PSUM tile. Col 255 (a level-8 node, recomputed per
    tile later) is included only to keep N>=256: f32r matmuls with a moving
    dim under 256 pay a 4x per-row penalty."""
    ps = psp.tile([128, 2048], F32, tag="ps")
    for i in range(4):
        bt = half * 4 + i
        nc.tensor.matmul(
            ps[:, i * 256:(i + 1) * 256],
            xt_r[:, bt * 128:(bt + 1) * 128],
            wdt_r[:, 0:256],
            start=True, stop=True,
        )
    return ps


def _phase_a_tree(nc, s_A, vA, vB, half):
    """Levels 0..7 batched over 4 tiles: v8 half [128, 4, 256]."""
    MUL = mybir.AluOpType.mult
    ADD = mybir.AluOpType.add
    g = slice(half * 4, (half + 1) * 4)
    # level 0: v1 = [C*s0, C - C*s0]
    nc.vector.tensor_scalar_mul(vA[:, g, 0:1], s_A[:, g, 0:1], C_SCALE)
    nc.vector.tensor_scalar(vA[:, g, 1:2], s_A[:, g, 0:1], -C_SCALE, C_SCALE, MUL, ADD)
    cur, nxt = vA, vB
    for lvl in range(1, 8):
        n = 1 << lvl
        off = n - 1
        nc.vector.tensor_mul(nxt[:, g, 0:n], cur[:, g, 0:n], s_A[:, g, off:off + n])
        nc.vector.tensor_sub(nxt[:, g, n:2 * n], cur[:, g, 0:n], nxt[:, g, 0:n])
        cur, nxt = nxt, cur
    return cur  # [128, NBT, 256] buffer holding v_8 for this half


def _tile_deep(nc, psp, pb, wdt_r, xt_r, out, v8, bt, fine_tail=False):
    """Per batch tile: matmul cols 255..4094, sigmoid, levels 8..11, DMA.

    fine_tail splits the last sub+DMA into 1024-col pieces so the kernel's
    final DMA is 728ns instead of 1456ns (only worth it on the last tile).
    """
    SIG = mybir.ActivationFunctionType.Sigmoid
    xs = xt_r[:, bt * 128:(bt + 1) * 128]
    ps1 = psp.tile([128, 2048], F32, tag="ps")   # cols 255..2302
    for c in range(4):
        nc.tensor.matmul(
            ps1[:, c * 512:(c + 1) * 512],
            xs, wdt_r[:, PS1_LO + c * 512:PS1_LO + (c + 1) * 512],
            start=True, stop=True,
        )
    ps2 = psp.tile([128, 1792], F32, tag="ps")   # cols 2303..4094
    for c in range(4):
        w0 = PS2_LO + c * 512
        w1 = min(w0 + 512, PS2_HI)
        nc.tensor.matmul(
            ps2[:, c * 512:c * 512 + (w1 - w0)],
            xs, wdt_r[:, w0:w1],
            start=True, stop=True,
        )
    # sigma of cols 255..4094, split so level 8 unblocks early
    s_B = pb.tile([128, 3840], F16, tag="sB")
    nc.scalar.activation(out=s_B[:, 0:768], in_=ps1[:, 0:768], func=SIG)
    nc.scalar.activation(out=s_B[:, 768:2048], in_=ps1[:, 768:2048], func=SIG)
    nc.scalar.activation(out=s_B[:, 2048:3840], in_=ps2[:], func=SIG)

    v9 = pb.tile([128, 512], F16, tag="v9")
    nc.vector.tensor_mul(v9[:, 0:256], v8[:, bt, :], s_B[:, 0:256])
    nc.vector.tensor_sub(v9[:, 256:512], v8[:, bt, :], v9[:, 0:256])
    v10 = pb.tile([128, 1024], F16, tag="v10")
    nc.vector.tensor_mul(v10[:, 0:512], v9[:], s_B[:, 256:768])
    nc.vector.tensor_sub(v10[:, 512:1024], v9[:], v10[:, 0:512])
    v11 = pb.tile([128, 2048], F16, tag="v11")
    nc.vector.tensor_mul(v11[:, 0:1024], v10[:], s_B[:, 768:1792])
    nc.vector.tensor_sub(v11[:, 1024:2048], v10[:], v11[:, 0:1024])
    ot = pb.tile([128, 4096], F16, tag="out")
    rows = out[bt * 128:(bt + 1) * 128, :]
    nc.vector.tensor_mul(ot[:, 0:2048], v11[:], s_B[:, 1792:3840])
    nc.sync.dma_start(out=rows[:, 0:2048], in_=ot[:, 0:2048])
    if fine_tail:
        nc.vector.tensor_sub(ot[:, 2048:3072], v11[:, 0:1024], ot[:, 0:1024])
        nc.sync.dma_start(out=rows[:, 2048:3072], in_=ot[:, 2048:3072])
        nc.vector.tensor_sub(ot[:, 3072:4096], v11[:, 1024:2048], ot[:, 1024:2048])
        nc.sync.dma_start(out=rows[:, 3072:4096], in_=ot[:, 3072:4096])
    else:
        nc.vector.tensor_sub(ot[:, 2048:4096], v11[:], ot[:, 0:2048])
        nc.sync.dma_start(out=rows[:, 2048:4096], in_=ot[:, 2048:4096])


def _emit_body(nc, tc, pa, pb, psp, wdt_r, xt_r, out):
    SIG = mybir.ActivationFunctionType.Sigmoid
    s_A = pa.tile([128, NBT, 256], F16, tag="sA")
    vA = pa.tile([128, NBT, 256], F16, tag="vA")
    vB = pa.tile([128, NBT, 256], F16, tag="vB")

    psA1 = _phase_a_mm(nc, psp, wdt_r, xt_r, 0)
    nc.scalar.activation(
        out=s_A[:, 0:4, :].rearrange("p g n -> p (g n)"),
        in_=psA1[:, 0:1024], func=SIG,
    )
    psA2 = _phase_a_mm(nc, psp, wdt_r, xt_r, 1)
    nc.scalar.activation(
        out=s_A[:, 4:8, :].rearrange("p g n -> p (g n)"),
        in_=psA2[:, 0:1024], func=SIG,
    )
    v8_h1 = _phase_a_tree(nc, s_A, vA, vB, 0)

    _tile_deep(nc, psp, pb, wdt_r, xt_r, out, v8_h1, 0)
    _tile_deep(nc, psp, pb, wdt_r, xt_r, out, v8_h1, 1)
    # half-2 shallow tree lands on the DVE queue here, off the ramp-up path;
    # its result buffer is the same ping-pong pair, ending in the same tile.
    v8_h2 = _phase_a_tree(nc, s_A, vA, vB, 1)
    assert v8_h2 is v8_h1
    _tile_deep(nc, psp, pb, wdt_r, xt_r, out, v8_h1, 2)
    _tile_deep(nc, psp, pb, wdt_r, xt_r, out, v8_h1, 3)
    for bt in range(4, NBT):
        _tile_deep(nc, psp, pb, wdt_r, xt_r, out, v8_h2, bt,
                   fine_tail=(bt == NBT - 1))


_NC_CACHE = {}


def _get_nc(reps=1):
    if reps not in _NC_CACHE:
        _NC_CACHE[reps] = _build(reps)
    return _NC_CACHE[reps]


def _bitrev(i, bits):
    r = 0
    for _ in range(bits):
        r = (r << 1) | (i & 1)
        i >>= 1
    return r


def _node_perm():
    """ours-col -> tree level-order node, per the concat-layout position map.

    Position i at level l corresponds to path bits b_0..b_{l-1} with b_j at
    bit j of i; the level-order node index uses b_0 as MSB -> bitrev_l(i).
    """
    perm = np.empty(NODES, dtype=np.int64)
    for lvl in range(DEPTH):
        off = (1 << lvl) - 1
        for i in range(1 << lvl):
            perm[off + i] = off + _bitrev(i, lvl)
    return perm


_NODE_PERM = _node_perm()
# leaf L lives at raw position bitrev12(L)
_LEAF_PERM = np.array([_bitrev(j, DEPTH) for j in range(LEAVES)], dtype=np.int64)


def _prep_inputs(x, W, b):
    x = np.asarray(x, dtype=np.float32)
    W = np.asarray(W, dtype=np.float32)
    b = np.asarray(b, dtype=np.float32)
    Wd = (W[:, 0, :] - W[:, 1, :])[_NODE_PERM]   # [4095, 64] in ours-col order
    bd = (b[:, 0] - b[:, 1])[_NODE_PERM]         # [4095]
    wdt = np.zeros((KA, LEAVES), dtype=np.float32)
    wdt[:D, :NODES] = Wd.T
    wdt[D, :NODES] = bd
    xt = np.empty((KA, B), dtype=np.float32)
    xt[:D] = x.T
    xt[D] = 1.0
    in_maps = [
        {"wdt": wdt, "xt": np.ascontiguousarray(xt[:, c * BLOC:(c + 1) * BLOC])}
        for c in range(NCORES)
    ]
    return in_maps


def kernel(x, W, b):
    in_maps = _prep_inputs(x, W, b)
    nc = _get_nc()
    res = run_bass_kernel_spmd(nc, in_maps, core_ids=list(range(NCORES)))
    raw = np.concatenate([res.results[c]["out"] for c in range(NCORES)], axis=0)
    return raw[:, _LEAF_PERM].astype(np.float32) * np.float32(1.0 / C_SCALE)


if __name__ == "__main__":
    rng = np.random.default_rng(0)
    x = rng.standard_normal((B, D)).astype(np.float32)
    W = (rng.standard_normal((NODES, 2, D)) * 0.1).astype(np.float32)
    b = (rng.standard_normal((NODES, 2)) * 0.1).astype(np.float32)
    p = kernel(x, W, b)
    print("out", p.shape, p.dtype, "rowsum", p.sum(axis=1)[:4])
